# revision 1
# baseline (speedup 1.0000x reference)
"""GAT (3-layer, DGL-style) on 8 Trainium2 NeuronCores.

Sharding: nodes across the 8 cores (6250 each, padded to 6272 = 49*128),
per-core nodes permuted by descending in-degree.  A "window" is 128 nodes;
a node is pinned to one SBUF partition lane of its window.  Per layer:

  Phase A (node side): featT = W^T @ h^T per window on PE, el/er via a small
  second matmul, build gather-table rows [feat(128 f32) | el(H f32)] with a
  768B stride in local DRAM, AllGather the tables across cores.

  Phase B (edge side): per window, edge tiles of 128 edges = one in-edge per
  destination partition.  dma_gather fetches 768B source rows (int16 indices;
  the 50176-row table is indexed as two 25088-row halves, each window's tiles
  are grouped into lo-half then hi-half passes).  er[dst] is a per-partition
  constant.  exp(lrelu(s)-C) = max(exp(s-C), exp(0.2*s-C)) on ACT.  Messages
  (+ per-head exp columns) are segment-summed by an identity-lhsT PE matmul
  accumulating into one PSUM bank per window.

C is a per-core bound lrelu(max el + max er) + 3 computed on device; shifting
exp by C instead of the per-segment max changes the reference's +1e-9 epsilon
term by < 1e-3 relative.
"""

import os
import sys

sys.path.insert(0, "/opt/trn_rl_repo")

import numpy as np

import concourse.bass as bass
import concourse.bacc as bacc
import concourse.mybir as mybir
import concourse.tile as tile
from concourse import library_config
from concourse.bass_utils import run_bass_kernel_spmd

F32 = mybir.dt.float32
I16 = mybir.dt.int16
AF = mybir.ActivationFunctionType
OP = mybir.AluOpType
AX = mybir.AxisListType

N_CORES = 8
DIM = 128
ROW_F32 = 192          # table row stride in f32 (768 B, multiple of 256 B)
TBL_COLS = 132         # used cols: 128 feat + up to 4 el slots
CAP = 16               # max tiles per dma_gather call
NEG_SLOPE = 0.2
C_MARGIN = 3.0
HEADS = (4, 4, 1)


# ---------------------------------------------------------------------------
# Host-side preprocessing
# ---------------------------------------------------------------------------

def preprocess(src, dst, n_nodes):
    src = np.asarray(src).astype(np.int64)
    dst = np.asarray(dst).astype(np.int64)
    npc = n_nodes // N_CORES
    NP = ((npc + 127) // 128) * 128
    W = NP // 128
    HALF = 4 * NP
    assert HALF <= 32768, HALF

    core = dst // npc
    local = dst - core * npc

    perm = []
    pos_of = np.empty(n_nodes, dtype=np.int64)
    for c in range(N_CORES):
        deg_c = np.bincount(local[core == c], minlength=npc)
        p = np.argsort(-deg_c, kind="stable")
        perm.append(p)
        inv = np.empty(npc, dtype=np.int64)
        inv[p] = np.arange(npc)
        pos_of[c * npc:(c + 1) * npc] = inv
    row_of = (np.arange(n_nodes) // npc) * NP + pos_of

    seg_pos = pos_of[dst]
    wv = seg_pos // 128
    pv = seg_pos % 128
    half = (row_of[src] >= HALF).astype(np.int64)

    # occurrence rank within (core, seg, half)
    key = (core * NP + seg_pos) * 2 + half
    order = np.argsort(key, kind="stable")
    ks = key[order]
    starts = np.r_[0, np.flatnonzero(np.diff(ks)) + 1]
    gid = np.zeros(len(ks), dtype=np.int64)
    gid[starts[1:]] = 1
    gid = np.cumsum(gid)
    t_in = np.arange(len(ks)) - starts[gid]
    tv = np.empty(len(ks), dtype=np.int64)
    tv[order] = t_in

    cnt = np.bincount(key, minlength=N_CORES * NP * 2).reshape(
        N_CORES, W, 128, 2)
    T_lo = cnt[:, :, :, 0].max(axis=(0, 2)).astype(np.int64)
    T_hi = cnt[:, :, :, 1].max(axis=(0, 2)).astype(np.int64)

    calls = []
    for w in range(W):
        for hf, T in ((0, int(T_lo[w])), (1, int(T_hi[w]))):
            t = 0
            while t < T:
                nt = min(CAP, T - t)
                calls.append((w, hf, nt))
                t += nt
    gtot = int(T_lo.sum() + T_hi.sum())
    icols = 8 * sum(nt for (_, _, nt) in calls)

    tile_off = np.zeros((W, 2), dtype=np.int64)
    acc = 0
    for w in range(W):
        tile_off[w, 0] = acc
        acc += T_lo[w]
        tile_off[w, 1] = acc
        acc += T_hi[w]

    idx_imgs, valids = [], []
    for c in range(N_CORES):
        m = core == c
        slots_idx = np.zeros((128, gtot), dtype=np.int64)
        slots_val = np.zeros((128, gtot), dtype=np.float32)
        g = tile_off[wv[m], half[m]] + tv[m]
        slots_idx[pv[m], g] = row_of[src[m]] - half[m] * HALF
        slots_val[pv[m], g] = 1.0
        img = np.zeros((16, icols), dtype=np.int16)
        colp = 0
        tile_ptr = {}
        for (w, hf, nt) in calls:
            t0 = tile_ptr.get((w, hf), 0)
            g0 = tile_off[w, hf] + t0
            part = slots_idx[:, g0:g0 + nt]          # [128, nt]
            flat = part.T.reshape(-1)                # j = t*128 + p
            img[:, colp:colp + nt * 8] = flat.reshape(nt * 8, 16).T
            colp += nt * 8
            tile_ptr[(w, hf)] = t0 + nt
        idx_imgs.append(np.ascontiguousarray(np.tile(img, (8, 1))))
        valids.append(slots_val)

    return dict(perm=perm, calls=calls, T_lo=T_lo, T_hi=T_hi,
                idx_img=idx_imgs, valid=valids, NP=NP, W=W, gtot=gtot,
                icols=icols, npc=npc, HALF=HALF,
                tile_off=tile_off)


def pack_weights(Wl, al, ar):
    H, Dh = Wl.shape[1], Wl.shape[2]
    Wm = np.ascontiguousarray(np.asarray(Wl, dtype=np.float32)
                              .reshape(Wl.shape[0], H * Dh))
    A = np.zeros((H * Dh, 8), dtype=np.float32)
    for h in range(H):
        A[h * Dh:(h + 1) * Dh, h] = np.asarray(al, dtype=np.float32)[h]
        A[h * Dh:(h + 1) * Dh, 4 + h] = np.asarray(ar, dtype=np.float32)[h]
    return Wm, A


# ---------------------------------------------------------------------------
# Device kernel
# ---------------------------------------------------------------------------

def build_nc(meta):
    NP, W, gtot, icols = meta["NP"], meta["W"], meta["gtot"], meta["icols"]
    calls, HALF = meta["calls"], meta["HALF"]
    NTOT = N_CORES * NP
    tile_off = meta["tile_off"]

    nc = bacc.Bacc(None, target_bir_lowering=False, debug=False,
                   num_devices=N_CORES, num_swdge_queues=4)

    hT0 = nc.declare_dram_parameter("hT0", [128, NP], F32, isOutput=False)
    idx_p = nc.declare_dram_parameter("idx", [128, icols], I16, isOutput=False)
    val_p = nc.declare_dram_parameter("valid", [128, gtot], F32,
                                      isOutput=False)
    Wp = [nc.declare_dram_parameter(f"W{l}", [128, 128], F32, isOutput=False)
          for l in range(3)]
    Ap = [nc.declare_dram_parameter(f"A{l}", [128, 8], F32, isOutput=False)
          for l in range(3)]
    ident_p = nc.declare_dram_parameter("ident", [128, 128], F32,
                                        isOutput=False)
    ones_p = nc.declare_dram_parameter("ones1", [1, 128], F32, isOutput=False)
    onescol_p = nc.declare_dram_parameter("onescol", [128, 1], F32,
                                          isOutput=False)
    out_p = nc.declare_dram_parameter("out", [NP, 128], F32, isOutput=True)

    with tile.TileContext(nc) as tc:
        with (
            tc.tile_pool(name="const", bufs=1) as constp,
            tc.tile_pool(name="persist", bufs=1) as pers,
            tc.tile_pool(name="featg", bufs=3) as fgp,
            tc.tile_pool(name="mext", bufs=3) as mxp,
            tc.tile_pool(name="small", bufs=4) as smp,
            tc.tile_pool(name="psum", bufs=3, space="PSUM") as psp,
            tc.tile_pool(name="psacc", bufs=2, space="PSUM") as psaccp,
            tc.tile_pool(name="dram", bufs=1, space="DRAM") as dramp,
        ):
            ident = constp.tile([128, 128], F32, tag="ident")
            nc.sync.dma_start(ident[:], ident_p[:, :])
            ones1 = constp.tile([1, 128], F32, tag="ones1")
            nc.sync.dma_start(ones1[:], ones_p[:, :])
            onescol = constp.tile([128, 1], F32, tag="onescol")
            nc.sync.dma_start(onescol[:], onescol_p[:, :])
            Wt = [constp.tile([128, 128], F32, tag=f"W{l}", name=f"Wt{l}") for l in range(3)]
            At = [constp.tile([128, 8], F32, tag=f"A{l}", name=f"At{l}") for l in range(3)]
            for l in range(3):
                nc.sync.dma_start(Wt[l][:], Wp[l][:, :])
                nc.sync.dma_start(At[l][:], Ap[l][:, :])
            idx_sb = pers.tile([128, icols], I16, tag="idx")
            nc.sync.dma_start(idx_sb[:], idx_p[:, :])
            valid_sb = pers.tile([128, gtot], F32, tag="valid")
            nc.sync.dma_start(valid_sb[:], val_p[:, :])

            hT = [pers.tile([128, W, 128], F32, tag=f"hT{i}", name=f"hT{i}")
                  for i in range(2)]
            nc.sync.dma_start(hT[0][:, :, :],
                              hT0[:, :].rearrange("p (w n) -> p w n", w=W))

            elerB = pers.tile([128, W, 8], F32, tag="elerB")
            rowimg = pers.tile([128, W, TBL_COLS], F32, tag="rowimg")

            loc_tbl = dramp.tile([NP, ROW_F32], F32, tag="loctbl")
            full_tbl = dramp.tile([NTOT, ROW_F32], F32, tag="fulltbl")
            zpad = smp.tile([128, ROW_F32 - TBL_COLS], F32, tag="zpad")
            nc.vector.memset(zpad[:], 0.0)
            for w in range(W):
                nc.sync.dma_start(
                    loc_tbl[:].rearrange("(w p) f -> w p f", p=128)
                    [w, :, TBL_COLS:ROW_F32],
                    zpad[:])


            CUT = os.environ.get("KGAT_CUT", "")
            n_layers = 1 if CUT else 3
            for layer in range(n_layers):
                H = HEADS[layer]
                D = 128 // H
                hcur, hnext = hT[layer % 2], hT[(layer + 1) % 2]

                # ======== Phase A ========
                if CUT == "B":
                    nc.vector.memset(rowimg[:, :, 0:TBL_COLS], 0.5)
                    nc.vector.memset(elerB[:, :, :], 0.1)
                for w in ([] if CUT == "B" else range(W)):
                    featT_ps = psp.tile([128, 128], F32, tag="ps")
                    nc.tensor.matmul(featT_ps[:], Wt[layer][:],
                                     hcur[:, w, :], start=True, stop=True)
                    featT_sb = smp.tile([128, 128], F32, tag="featT_sb")
                    nc.vector.tensor_copy(featT_sb[:], featT_ps[:])
                    elerT_ps = psp.tile([8, 128], F32, tag="ps")
                    nc.tensor.matmul(elerT_ps[:], At[layer][:], featT_sb[:],
                                     start=True, stop=True)
                    elerT_sb = smp.tile([8, 128], F32, tag="elerT_sb")
                    nc.vector.tensor_copy(elerT_sb[:], elerT_ps[:])
                    eler_ps = psp.tile([128, 8], F32, tag="ps")
                    nc.tensor.matmul(eler_ps[:], elerT_sb[:],
                                     ident[0:8, 0:8], is_transpose=True,
                                     start=True, stop=True)
                    nc.vector.tensor_copy(elerB[:, w, :], eler_ps[:])
                    feat_ps = psp.tile([128, 128], F32, tag="ps")
                    nc.tensor.matmul(feat_ps[:], featT_sb[:], ident[:, :],
                                     is_transpose=True, start=True, stop=True)
                    nc.vector.tensor_copy(rowimg[:, w, 0:128], feat_ps[:])
                    nc.vector.tensor_copy(rowimg[:, w, 128:128 + H],
                                          eler_ps[:, 0:H])
                    nc.sync.dma_start(
                        loc_tbl[:].rearrange("(w p) f -> w p f", p=128)
                        [w, :, 0:TBL_COLS],
                        rowimg[:, w, :])
                if CUT == "B":
                    for w in range(W):
                        nc.sync.dma_start(
                            loc_tbl[:].rearrange("(w p) f -> w p f", p=128)
                            [w, :, 0:TBL_COLS],
                            rowimg[:, w, :])

                # ---- AllGather ----
                nc.gpsimd.collective_compute(
                    "AllGather", OP.bypass,
                    replica_groups=[list(range(N_CORES))],
                    ins=[loc_tbl[:].opt()], outs=[full_tbl[:].opt()])

                # ---- -C = -(lrelu(max el + max er) + margin) ----
                if CUT == "B":
                    negC = smp.tile([128, 1], F32, tag="negC")
                    nc.vector.memset(negC[:], -1.0)
                else:
                    mx = smp.tile([128, 2], F32, tag="mx")
                    nc.vector.tensor_reduce(mx[:, 0:1], elerB[:, :, 0:H],
                                            axis=AX.XY, op=OP.max)
                    nc.vector.tensor_reduce(mx[:, 1:2], elerB[:, :, 4:4 + H],
                                            axis=AX.XY, op=OP.max)
                    mxT_ps = psp.tile([2, 128], F32, tag="ps")
                    nc.tensor.matmul(mxT_ps[:], mx[:], ident[:, :],
                                     is_transpose=True, start=True, stop=True)
                    mm = smp.tile([2, 1], F32, tag="mm")
                    nc.vector.tensor_reduce(mm[:], mxT_ps[:, :], axis=AX.X,
                                            op=OP.max)
                    s_ps = psp.tile([1, 1], F32, tag="ps")
                    nc.tensor.matmul(s_ps[:], mm[:], onescol[0:2, 0:1],
                                     start=True, stop=True)
                    cs = smp.tile([1, 4], F32, tag="cs")
                    nc.vector.tensor_copy(cs[:, 0:1], s_ps[:])
                    nc.vector.tensor_scalar(cs[:, 1:2], cs[:, 0:1], NEG_SLOPE,
                                            None, op0=OP.mult)
                    nc.vector.tensor_tensor(cs[:, 2:3], cs[:, 0:1],
                                            cs[:, 1:2], op=OP.max)
                    nc.vector.tensor_scalar(cs[:, 3:4], cs[:, 2:3], -1.0,
                                            -C_MARGIN, op0=OP.mult,
                                            op1=OP.add)
                    negC_ps = psp.tile([128, 1], F32, tag="ps")
                    nc.tensor.matmul(negC_ps[:], ones1[:], cs[:, 3:4],
                                     start=True, stop=True)
                    negC = smp.tile([128, 1], F32, tag="negC")
                    nc.vector.tensor_copy(negC[:], negC_ps[:])

                # ======== Phase B ========
                tbl_lo = full_tbl[0:HALF, :]
                tbl_hi = full_tbl[HALF:NTOT, :]
                colp = 0
                tile_ptr = {}
                cur_w = -1
                acc_ps = None
                first_mm = True
                ntiles_w = {w: int(meta["T_lo"][w] + meta["T_hi"][w])
                            for w in range(W)}
                done_w = {w: 0 for w in range(W)}
                qn = 0
                for (w, hf, nt) in (calls if CUT != "A" else []):
                    if w != cur_w:
                        cur_w = w
                        acc_ps = psaccp.tile([128, TBL_COLS], F32, tag="acc")
                        first_mm = True
                    t0 = tile_ptr.get((w, hf), 0)
                    tile_ptr[(w, hf)] = t0 + nt
                    g0 = int(tile_off[w, hf]) + t0

                    fg = fgp.tile([128, CAP, ROW_F32], F32, tag="fg")
                    src_ap = tbl_lo if hf == 0 else tbl_hi
                    nc.gpsimd.dma_gather(
                        fg[:, 0:nt, :], src_ap,
                        idx_sb[:, colp:colp + nt * 8],
                        nt * 128, nt * 128, ROW_F32, elem_step=ROW_F32,
                        single_packet=False, queue_num=qn)
                    qn = (qn + 1) % 4
                    colp += nt * 8

                    t = 0
                    while t < nt and CUT not in ("AB", "B"):
                        g = min(4, nt - t)
                        sx = smp.tile([128, 4, 4], F32, tag="sx")
                        ux = smp.tile([128, 4, 4], F32, tag="ux")
                        ex = smp.tile([128, 4, 4], F32, tag="exx")
                        er_b = (elerB[:, w, 4:4 + H].unsqueeze(1)
                                .broadcast_to([128, g, H]))
                        nc.vector.tensor_tensor(
                            sx[:, 0:g, 0:H], fg[:, t:t + g, 128:128 + H],
                            er_b, op=OP.add)
                        nc.scalar.activation(ux[:, 0:g, 0:H], sx[:, 0:g, 0:H],
                                             AF.Exp, bias=negC[:, 0:1],
                                             scale=1.0)
                        nc.scalar.activation(ex[:, 0:g, 0:H], sx[:, 0:g, 0:H],
                                             AF.Exp, bias=negC[:, 0:1],
                                             scale=NEG_SLOPE)
                        val_b = (valid_sb[:, g0 + t:g0 + t + g].unsqueeze(2)
                                 .broadcast_to([128, g, H]))
                        nc.vector.scalar_tensor_tensor(
                            ex[:, 0:g, 0:H], ux[:, 0:g, 0:H], 1.0,
                            ex[:, 0:g, 0:H], op0=OP.mult, op1=OP.max)
                        nc.vector.tensor_tensor(ex[:, 0:g, 0:H],
                                                ex[:, 0:g, 0:H], val_b,
                                                op=OP.mult)
                        mext = mxp.tile([128, 4, TBL_COLS], F32, tag="mext")
                        ex_b = (ex[:, 0:g, 0:H].unsqueeze(3)
                                .broadcast_to([128, g, H, D]))
                        nc.vector.tensor_tensor(
                            mext[:, 0:g, 0:128]
                            .rearrange("p g (h d) -> p g h d", h=H),
                            fg[:, t:t + g, 0:128]
                            .rearrange("p g (h d) -> p g h d", h=H),
                            ex_b, op=OP.mult)
                        nc.vector.tensor_copy(mext[:, 0:g, 128:128 + H],
                                              ex[:, 0:g, 0:H])
                        for k in range(g):
                            done_w[w] += 1
                            nc.tensor.matmul(
                                acc_ps[:, 0:128 + H], ident[:, :],
                                mext[:, k, 0:128 + H],
                                start=first_mm,
                                stop=(done_w[w] == ntiles_w[w]))
                            first_mm = False
                        t += g

                    if CUT in ("AB", "ABC") and tile_ptr[(w, hf)] >= 0:
                        pass
                    if done_w[w] == ntiles_w[w] and not CUT:
                        dn = smp.tile([128, 8], F32, tag="dn")
                        nc.vector.tensor_scalar(dn[:, 0:H],
                                                acc_ps[:, 128:128 + H],
                                                1e-9, None, op0=OP.add)
                        nc.vector.reciprocal(dn[:, 4:4 + H], dn[:, 0:H])
                        hsb = smp.tile([128, 128], F32, tag="hsb")
                        rec_b = (dn[:, 4:4 + H].unsqueeze(2)
                                 .broadcast_to([128, H, D]))
                        nc.vector.tensor_tensor(
                            hsb[:].rearrange("p (h d) -> p h d", h=H),
                            acc_ps[:, 0:128]
                            .rearrange("p (h d) -> p h d", h=H),
                            rec_b, op=OP.mult)
                        if layer < 2:
                            hT_ps = psp.tile([128, 128], F32, tag="ps")
                            nc.tensor.matmul(hT_ps[:], hsb[:], ident[:, :],
                                             is_transpose=True,
                                             start=True, stop=True)
                            nc.scalar.activation(hnext[:, w, :], hT_ps[:],
                                                 AF.Relu)
                        else:
                            nc.sync.dma_start(
                                out_p[:, :].rearrange("(w p) f -> w p f",
                                                      p=128)[w, :, :],
                                hsb[:])
            if CUT:
                for w in range(W):
                    nc.sync.dma_start(
                        out_p[:, :].rearrange("(w p) f -> w p f", p=128)
                        [w, :, :],
                        rowimg[:, w, 0:128])
    nc.finalize()
    return nc


# ---------------------------------------------------------------------------
# Entry point
# ---------------------------------------------------------------------------

def kernel(features, src, dst, W0, al0, ar0, W1, al1, ar1, W2, al2, ar2):
    out, _ = run_gat(features, src, dst, W0, al0, ar0, W1, al1, ar1,
                     W2, al2, ar2, trace=False)
    return out


def run_gat(features, src, dst, W0, al0, ar0, W1, al1, ar1, W2, al2, ar2,
            trace=False):
    features = np.asarray(features, dtype=np.float32)
    n_nodes = features.shape[0]
    meta = preprocess(src, dst, n_nodes)
    NP, W, npc = meta["NP"], meta["W"], meta["npc"]

    Wm0, A0 = pack_weights(np.asarray(W0), al0, ar0)
    Wm1, A1 = pack_weights(np.asarray(W1), al1, ar1)
    Wm2, A2 = pack_weights(np.asarray(W2), al2, ar2)

    ident = np.eye(128, dtype=np.float32)
    ones1 = np.ones((1, 128), dtype=np.float32)
    onescol = np.ones((128, 1), dtype=np.float32)

    in_maps = []
    for c in range(N_CORES):
        h_c = np.zeros((NP, 128), dtype=np.float32)
        h_c[:npc] = features[c * npc:(c + 1) * npc][meta["perm"][c]]
        in_maps.append({
            "hT0": np.ascontiguousarray(h_c.T),
            "idx": meta["idx_img"][c],
            "valid": meta["valid"][c],
            "W0": Wm0, "W1": Wm1, "W2": Wm2,
            "A0": A0, "A1": A1, "A2": A2,
            "ident": ident, "ones1": ones1, "onescol": onescol,
        })

    nc = build_nc(meta)
    br = run_bass_kernel_spmd(nc, in_maps, list(range(N_CORES)), trace=trace)
    res = br.results

    out = np.empty((n_nodes, 128), dtype=np.float32)
    for c in range(N_CORES):
        o = np.asarray(res[c]["out"])
        out[c * npc:(c + 1) * npc] = o[np.argsort(meta["perm"][c])]
    return out, br



# revision 4
# speedup vs baseline: 1.0799x; 1.0799x over previous
"""GAT (3-layer, DGL-style) on 8 Trainium2 NeuronCores — v2 (bf16 tables).

Sharding: nodes across the 8 cores (6250 each, padded to 6272 = 49*128),
per-core nodes permuted by descending in-degree.  A "window" is 128 nodes;
a node is pinned to one SBUF partition lane of its window.  Per layer:

  Phase A (node side): featT = W^T @ h^T per window on PE (bf16), el/er via a
  small second matmul, build 512-byte gather-table rows
  [feat 128xbf16 | el 4xf32-bitcast | pad] in local DRAM.  The table is
  AllGathered in 4 window-quarter chunks interleaved with phase A so most of
  the collective hides behind compute.

  Phase B (edge side): per window, edge tiles of 128 edges = one in-edge per
  destination partition.  dma_gather fetches 512B source rows (int16 indices;
  the quarter-major table row order makes rows 0..24575 the "lo" half and
  24576..50175 the "hi" half, each addressable with int16).  All windows' lo
  tiles run first (partial sums flushed PSUM->SBUF), then all hi tiles (so
  the hi-half AllGather chunks hide behind the whole lo pass).  er[dst] is a
  per-partition constant.  exp(lrelu(s)-C) = max(exp(s-C), exp(0.2*s-C)) on
  ACT.  Messages (+ per-head exp columns, all bf16) are segment-summed by a
  bf16 identity-lhsT PE matmul accumulating into one PSUM bank per window.

C is a per-core bound lrelu(max el + max er) + 3 computed on device; shifting
exp by C instead of the per-segment max changes the reference's +1e-9 epsilon
term by < 1e-3 relative.
"""

import sys

sys.path.insert(0, "/opt/trn_rl_repo")

import numpy as np
import ml_dtypes

import concourse.bass as bass
import concourse.bacc as bacc
import concourse.mybir as mybir
import concourse.tile as tile
from concourse.bass_utils import run_bass_kernel_spmd

F32 = mybir.dt.float32
BF16 = mybir.dt.bfloat16
I16 = mybir.dt.int16
AF = mybir.ActivationFunctionType
OP = mybir.AluOpType
AX = mybir.AxisListType

NPBF = ml_dtypes.bfloat16

N_CORES = 8
DIM = 128
ROW_BF = 256           # table row stride in bf16 elems (512 B)
TBL_COLS = 136         # used cols: 128 feat bf16 + 4 el f32 (8 bf16 slots)
CAP = 16               # max tiles per dma_gather call
GRP = 8                # tiles per DVE/ACT group
NEG_SLOPE = 0.2
C_MARGIN = 3.0
HEADS = (4, 4, 1)

# window-quarter split for chunked AllGather (49 windows)
QWIN = (12, 12, 12, 13)


def _quarter_geometry(NP, W):
    qsize = [q * 128 for q in QWIN]                  # per-core rows per chunk
    qstart = np.cumsum([0] + qsize[:-1]).tolist()    # per-core pos offsets
    chunk_off = np.cumsum([0] + [N_CORES * s for s in qsize[:-1]]).tolist()
    return qsize, qstart, chunk_off


# ---------------------------------------------------------------------------
# Host-side preprocessing
# ---------------------------------------------------------------------------

def preprocess(src, dst, n_nodes):
    src = np.asarray(src).astype(np.int64)
    dst = np.asarray(dst).astype(np.int64)
    npc = n_nodes // N_CORES
    NP = ((npc + 127) // 128) * 128
    W = NP // 128
    assert W == sum(QWIN)
    qsize, qstart, chunk_off = _quarter_geometry(NP, W)
    HALF = chunk_off[2]
    assert HALF <= 32768 and (N_CORES * NP - HALF) <= 32768

    core = dst // npc
    local = dst - core * npc

    perm = []
    pos_of = np.empty(n_nodes, dtype=np.int64)
    for c in range(N_CORES):
        deg_c = np.bincount(local[core == c], minlength=npc)
        p = np.argsort(-deg_c, kind="stable")
        perm.append(p)
        inv = np.empty(npc, dtype=np.int64)
        inv[p] = np.arange(npc)
        pos_of[c * npc:(c + 1) * npc] = inv

    # quarter-major global table row for each node
    posq = pos_of  # per-core position 0..NP-1 (only first npc used per core)
    qidx = np.searchsorted(np.array(qstart[1:] + [NP]), posq, side="right")
    qs = np.array([qstart[q] for q in range(4)])[qidx]
    qz = np.array([qsize[q] for q in range(4)])[qidx]
    co = np.array([chunk_off[q] for q in range(4)])[qidx]
    node_core = np.arange(n_nodes) // npc
    row_of = co + node_core * qz + (posq - qs)

    seg_pos = pos_of[dst]
    wv = seg_pos // 128
    pv = seg_pos % 128
    half = (row_of[src] >= HALF).astype(np.int64)

    # occurrence rank within (core, seg, half)
    key = (core * NP + seg_pos) * 2 + half
    order = np.argsort(key, kind="stable")
    ks = key[order]
    starts = np.r_[0, np.flatnonzero(np.diff(ks)) + 1]
    gid = np.zeros(len(ks), dtype=np.int64)
    gid[starts[1:]] = 1
    gid = np.cumsum(gid)
    t_in = np.arange(len(ks)) - starts[gid]
    tv = np.empty(len(ks), dtype=np.int64)
    tv[order] = t_in

    cnt = np.bincount(key, minlength=N_CORES * NP * 2).reshape(
        N_CORES, W, 128, 2)
    T_lo = cnt[:, :, :, 0].max(axis=(0, 2)).astype(np.int64)
    T_hi = cnt[:, :, :, 1].max(axis=(0, 2)).astype(np.int64)
    assert (T_lo > 0).all() and (T_hi > 0).all()

    # lo pass over all windows first, then hi pass
    calls = []
    for hf, Tarr in ((0, T_lo), (1, T_hi)):
        for w in range(W):
            t = 0
            T = int(Tarr[w])
            while t < T:
                nt = min(CAP, T - t)
                calls.append((w, hf, nt))
                t += nt
    gtot = int(T_lo.sum() + T_hi.sum())
    icols = 8 * sum(nt for (_, _, nt) in calls)

    tile_off = np.zeros((W, 2), dtype=np.int64)
    acc = 0
    for w in range(W):
        tile_off[w, 0] = acc
        acc += T_lo[w]
        tile_off[w, 1] = acc
        acc += T_hi[w]

    idx_imgs, valids = [], []
    for c in range(N_CORES):
        m = core == c
        slots_idx = np.zeros((128, gtot), dtype=np.int64)
        slots_val = np.zeros((128, gtot), dtype=np.float32)
        g = tile_off[wv[m], half[m]] + tv[m]
        slots_idx[pv[m], g] = row_of[src[m]] - half[m] * HALF
        slots_val[pv[m], g] = 1.0
        img = np.zeros((16, icols), dtype=np.int16)
        colp = 0
        tile_ptr = {}
        for (w, hf, nt) in calls:
            t0 = tile_ptr.get((w, hf), 0)
            g0 = tile_off[w, hf] + t0
            part = slots_idx[:, g0:g0 + nt]          # [128, nt]
            flat = part.T.reshape(-1)                # j = t*128 + p
            img[:, colp:colp + nt * 8] = flat.reshape(nt * 8, 16).T
            colp += nt * 8
            tile_ptr[(w, hf)] = t0 + nt
        idx_imgs.append(np.ascontiguousarray(np.tile(img, (8, 1))))
        valids.append(slots_val.astype(NPBF))

    return dict(perm=perm, calls=calls, T_lo=T_lo, T_hi=T_hi,
                idx_img=idx_imgs, valid=valids, NP=NP, W=W, gtot=gtot,
                icols=icols, npc=npc, HALF=HALF,
                tile_off=tile_off, qsize=qsize, qstart=qstart,
                chunk_off=chunk_off)


def pack_weights(Wl, al, ar):
    H, Dh = Wl.shape[1], Wl.shape[2]
    Wm = np.ascontiguousarray(np.asarray(Wl, dtype=np.float32)
                              .reshape(Wl.shape[0], H * Dh))
    A = np.zeros((H * Dh, 8), dtype=np.float32)
    for h in range(H):
        A[h * Dh:(h + 1) * Dh, h] = np.asarray(al, dtype=np.float32)[h]
        A[h * Dh:(h + 1) * Dh, 4 + h] = np.asarray(ar, dtype=np.float32)[h]
    return Wm.astype(NPBF), A.astype(NPBF)


# ---------------------------------------------------------------------------
# Device kernel
# ---------------------------------------------------------------------------

def build_nc(meta):
    NP, W, gtot, icols = meta["NP"], meta["W"], meta["gtot"], meta["icols"]
    calls, HALF = meta["calls"], meta["HALF"]
    NTOT = N_CORES * NP
    tile_off = meta["tile_off"]
    qsize, qstart, chunk_off = meta["qsize"], meta["qstart"], meta["chunk_off"]
    qlastw = np.cumsum(QWIN) - 1  # windows 11, 23, 35, 48

    nc = bacc.Bacc(None, target_bir_lowering=False, debug=False,
                   num_devices=N_CORES, num_swdge_queues=4)

    hT0 = nc.declare_dram_parameter("hT0", [128, NP], BF16, isOutput=False)
    idx_p = nc.declare_dram_parameter("idx", [128, icols], I16, isOutput=False)
    val_p = nc.declare_dram_parameter("valid", [128, gtot], BF16,
                                      isOutput=False)
    Wp = [nc.declare_dram_parameter(f"W{l}", [128, 128], BF16, isOutput=False)
          for l in range(3)]
    Ap = [nc.declare_dram_parameter(f"A{l}", [128, 8], BF16, isOutput=False)
          for l in range(3)]
    identF_p = nc.declare_dram_parameter("identF", [128, 128], F32,
                                         isOutput=False)
    identB_p = nc.declare_dram_parameter("identB", [128, 128], BF16,
                                         isOutput=False)
    ones_p = nc.declare_dram_parameter("ones1", [1, 128], F32, isOutput=False)
    onescol_p = nc.declare_dram_parameter("onescol", [128, 1], F32,
                                          isOutput=False)
    out_p = nc.declare_dram_parameter("out", [NP, 128], F32, isOutput=True)

    with tile.TileContext(nc) as tc:
        with (
            tc.tile_pool(name="const", bufs=1) as constp,
            tc.tile_pool(name="persist", bufs=1) as pers,
            tc.tile_pool(name="featg", bufs=3) as fgp,
            tc.tile_pool(name="mext", bufs=3) as mxp,
            tc.tile_pool(name="small", bufs=4) as smp,
            tc.tile_pool(name="psum", bufs=3, space="PSUM") as psp,
            tc.tile_pool(name="psacc", bufs=2, space="PSUM") as psaccp,
            tc.tile_pool(name="dram", bufs=1, space="DRAM") as dramp,
        ):
            identF = constp.tile([128, 128], F32, tag="identF")
            nc.sync.dma_start(identF[:], identF_p[:, :])
            identB = constp.tile([128, 128], BF16, tag="identB")
            nc.sync.dma_start(identB[:], identB_p[:, :])
            ones1 = constp.tile([1, 128], F32, tag="ones1")
            nc.sync.dma_start(ones1[:], ones_p[:, :])
            onescol = constp.tile([128, 1], F32, tag="onescol")
            nc.sync.dma_start(onescol[:], onescol_p[:, :])
            Wt = [constp.tile([128, 128], BF16, tag=f"W{l}", name=f"Wt{l}")
                  for l in range(3)]
            At = [constp.tile([128, 8], BF16, tag=f"A{l}", name=f"At{l}")
                  for l in range(3)]
            for l in range(3):
                nc.sync.dma_start(Wt[l][:], Wp[l][:, :])
                nc.sync.dma_start(At[l][:], Ap[l][:, :])
            idx_sb = pers.tile([128, icols], I16, tag="idx")
            nc.sync.dma_start(idx_sb[:], idx_p[:, :])
            valid_sb = pers.tile([128, gtot], BF16, tag="valid")
            nc.sync.dma_start(valid_sb[:], val_p[:, :])

            hT = [pers.tile([128, W, 128], BF16, tag=f"hT{i}", name=f"hT{i}")
                  for i in range(2)]
            nc.sync.dma_start(hT[0][:, :, :],
                              hT0[:, :].rearrange("p (w n) -> p w n", w=W))

            elerB = pers.tile([128, W, 8], F32, tag="elerB")
            rowimg = pers.tile([128, W, TBL_COLS], BF16, tag="rowimg")
            accsb = pers.tile([128, W, 132], F32, tag="accsb")

            loc_tbl = dramp.tile([NP, ROW_BF], BF16, tag="loctbl")
            full_tbl = dramp.tile([NTOT, ROW_BF], BF16, tag="fulltbl")

            for layer in range(3):
                H = HEADS[layer]
                D = 128 // H
                hcur, hnext = hT[layer % 2], hT[(layer + 1) % 2]

                # ======== Phase A ========
                agq = 0
                for w in range(W):
                    featT_ps = psp.tile([128, 128], F32, tag="ps")
                    nc.tensor.matmul(featT_ps[:], Wt[layer][:],
                                     hcur[:, w, :], start=True, stop=True)
                    featT_sb = smp.tile([128, 128], BF16, tag="featT_sb")
                    nc.vector.tensor_copy(featT_sb[:], featT_ps[:])
                    elerT_ps = psp.tile([8, 128], F32, tag="ps")
                    nc.tensor.matmul(elerT_ps[:], At[layer][:], featT_sb[:],
                                     start=True, stop=True)
                    elerT_sb = smp.tile([8, 128], F32, tag="elerT_sb")
                    nc.vector.tensor_copy(elerT_sb[:], elerT_ps[:])
                    eler_ps = psp.tile([128, 8], F32, tag="ps")
                    nc.tensor.matmul(eler_ps[:], elerT_sb[:],
                                     identF[0:8, 0:8], is_transpose=True,
                                     start=True, stop=True)
                    nc.vector.tensor_copy(elerB[:, w, :], eler_ps[:])
                    feat_ps = psp.tile([128, 128], BF16, tag="psb")
                    nc.tensor.matmul(feat_ps[:], featT_sb[:], identB[:, :],
                                     is_transpose=True, start=True, stop=True)
                    nc.vector.tensor_copy(rowimg[:, w, 0:128], feat_ps[:])
                    nc.vector.tensor_copy(
                        rowimg[:, w, 128:TBL_COLS].bitcast(F32),
                        eler_ps[:, 0:4])
                    nc.sync.dma_start(
                        loc_tbl[:].rearrange("(w p) f -> w p f", p=128)
                        [w, :, 0:TBL_COLS],
                        rowimg[:, w, :])
                    if w == qlastw[agq]:
                        q = agq
                        nc.gpsimd.collective_compute(
                            "AllGather", OP.bypass,
                            replica_groups=[list(range(N_CORES))],
                            ins=[loc_tbl[qstart[q]:qstart[q] + qsize[q], :]
                                 .opt()],
                            outs=[full_tbl[chunk_off[q]:
                                           chunk_off[q] + N_CORES * qsize[q],
                                           :].opt()])
                        agq += 1

                # ---- -C = -(lrelu(max el + max er) + margin) ----
                mx = smp.tile([128, 2], F32, tag="mx")
                nc.vector.tensor_reduce(mx[:, 0:1], elerB[:, :, 0:H],
                                        axis=AX.XY, op=OP.max)
                nc.vector.tensor_reduce(mx[:, 1:2], elerB[:, :, 4:4 + H],
                                        axis=AX.XY, op=OP.max)
                mxT_ps = psp.tile([2, 128], F32, tag="ps")
                nc.tensor.matmul(mxT_ps[:], mx[:], identF[:, :],
                                 is_transpose=True, start=True, stop=True)
                mm = smp.tile([2, 1], F32, tag="mm")
                nc.vector.tensor_reduce(mm[:], mxT_ps[:, :], axis=AX.X,
                                        op=OP.max)
                s_ps = psp.tile([1, 1], F32, tag="ps")
                nc.tensor.matmul(s_ps[:], mm[:], onescol[0:2, 0:1],
                                 start=True, stop=True)
                cs = smp.tile([1, 4], F32, tag="cs")
                nc.vector.tensor_copy(cs[:, 0:1], s_ps[:])
                nc.vector.tensor_scalar(cs[:, 1:2], cs[:, 0:1], NEG_SLOPE,
                                        None, op0=OP.mult)
                nc.vector.tensor_tensor(cs[:, 2:3], cs[:, 0:1],
                                        cs[:, 1:2], op=OP.max)
                nc.vector.tensor_scalar(cs[:, 3:4], cs[:, 2:3], -1.0,
                                        -C_MARGIN, op0=OP.mult,
                                        op1=OP.add)
                negC_ps = psp.tile([128, 1], F32, tag="ps")
                nc.tensor.matmul(negC_ps[:], ones1[:], cs[:, 3:4],
                                 start=True, stop=True)
                negC = smp.tile([128, 1], F32, tag="negC")
                nc.vector.tensor_copy(negC[:], negC_ps[:])

                # zero the lo-pass partial sums (covers any empty windows)
                nc.vector.memset(accsb[:, :, :], 0.0)

                # ======== Phase B (lo pass over all windows, then hi) ====
                tbl_lo = full_tbl[0:HALF, :]
                tbl_hi = full_tbl[HALF:NTOT, :]
                colp = 0
                tile_ptr = {}
                cur_key = None
                acc_ps = None
                first_mm = True
                done = {}
                qn = 0
                for (w, hf, nt) in calls:
                    Thf = int((meta["T_lo"] if hf == 0 else meta["T_hi"])[w])
                    if (w, hf) != cur_key:
                        cur_key = (w, hf)
                        acc_ps = psaccp.tile([128, 132], F32, tag="acc")
                        first_mm = True
                        done[cur_key] = 0
                    t0 = tile_ptr.get((w, hf), 0)
                    tile_ptr[(w, hf)] = t0 + nt
                    g0 = int(tile_off[w, hf]) + t0

                    fg = fgp.tile([128, CAP, ROW_BF], BF16, tag="fg")
                    src_ap = tbl_lo if hf == 0 else tbl_hi
                    nc.gpsimd.dma_gather(
                        fg[:, 0:nt, :], src_ap,
                        idx_sb[:, colp:colp + nt * 8],
                        nt * 128, nt * 128, ROW_BF, elem_step=ROW_BF,
                        single_packet=False, queue_num=qn)
                    qn = (qn + 1) % 4
                    colp += nt * 8

                    t = 0
                    while t < nt:
                        g = min(GRP, nt - t)
                        sx = smp.tile([128, GRP, 4], F32, tag="sx")
                        ux = smp.tile([128, GRP, 4], BF16, tag="ux")
                        ex = smp.tile([128, GRP, 4], BF16, tag="exx")
                        er_b = (elerB[:, w, 4:4 + H].unsqueeze(1)
                                .broadcast_to([128, g, H]))
                        nc.vector.tensor_tensor(
                            sx[:, 0:g, 0:H],
                            fg[:, t:t + g, 128:128 + 2 * H].bitcast(F32),
                            er_b, op=OP.add)
                        nc.scalar.activation(ux[:, 0:g, 0:H], sx[:, 0:g, 0:H],
                                             AF.Exp, bias=negC[:, 0:1],
                                             scale=1.0)
                        nc.scalar.activation(ex[:, 0:g, 0:H], sx[:, 0:g, 0:H],
                                             AF.Exp, bias=negC[:, 0:1],
                                             scale=NEG_SLOPE)
                        val_b = (valid_sb[:, g0 + t:g0 + t + g].unsqueeze(2)
                                 .broadcast_to([128, g, H]))
                        nc.vector.scalar_tensor_tensor(
                            ex[:, 0:g, 0:H], ux[:, 0:g, 0:H], 1.0,
                            ex[:, 0:g, 0:H], op0=OP.mult, op1=OP.max)
                        nc.vector.tensor_tensor(ex[:, 0:g, 0:H],
                                                ex[:, 0:g, 0:H], val_b,
                                                op=OP.mult)
                        mext = mxp.tile([128, GRP, 132], BF16, tag="mext")
                        ex_b = (ex[:, 0:g, 0:H].unsqueeze(3)
                                .broadcast_to([128, g, H, D]))
                        nc.vector.tensor_tensor(
                            mext[:, 0:g, 0:128]
                            .rearrange("p g (h d) -> p g h d", h=H),
                            fg[:, t:t + g, 0:128]
                            .rearrange("p g (h d) -> p g h d", h=H),
                            ex_b, op=OP.mult)
                        nc.vector.tensor_copy(mext[:, 0:g, 128:128 + H],
                                              ex[:, 0:g, 0:H])
                        for k in range(g):
                            done[cur_key] += 1
                            nc.tensor.matmul(
                                acc_ps[:, 0:128 + H], identB[:, :],
                                mext[:, k, 0:128 + H],
                                start=first_mm,
                                stop=(done[cur_key] == Thf))
                            first_mm = False
                        t += g

                    if done[cur_key] != Thf:
                        continue
                    if hf == 0:
                        # flush lo partial into SBUF
                        nc.vector.tensor_copy(accsb[:, w, 0:128 + H],
                                              acc_ps[:, 0:128 + H])
                        continue
                    # hi pass window epilogue: combine, normalize, emit
                    accf = smp.tile([128, 132], F32, tag="accf")
                    nc.vector.tensor_tensor(accf[:, 0:128 + H],
                                            acc_ps[:, 0:128 + H],
                                            accsb[:, w, 0:128 + H],
                                            op=OP.add)
                    dn = smp.tile([128, 8], F32, tag="dn")
                    nc.vector.tensor_scalar(dn[:, 0:H],
                                            accf[:, 128:128 + H],
                                            1e-9, None, op0=OP.add)
                    nc.vector.reciprocal(dn[:, 4:4 + H], dn[:, 0:H])
                    hsb = smp.tile([128, 128], F32, tag="hsb")
                    rec_b = (dn[:, 4:4 + H].unsqueeze(2)
                             .broadcast_to([128, H, D]))
                    nc.vector.tensor_tensor(
                        hsb[:].rearrange("p (h d) -> p h d", h=H),
                        accf[:, 0:128]
                        .rearrange("p (h d) -> p h d", h=H),
                        rec_b, op=OP.mult)
                    if layer < 2:
                        hT_ps = psp.tile([128, 128], F32, tag="ps")
                        nc.tensor.matmul(hT_ps[:], hsb[:], identF[:, :],
                                         is_transpose=True,
                                         start=True, stop=True)
                        nc.scalar.activation(hnext[:, w, :], hT_ps[:],
                                             AF.Relu)
                    else:
                        nc.sync.dma_start(
                            out_p[:, :].rearrange("(w p) f -> w p f",
                                                  p=128)[w, :, :],
                            hsb[:])
    nc.finalize()
    return nc


# ---------------------------------------------------------------------------
# Entry point
# ---------------------------------------------------------------------------

def kernel(features, src, dst, W0, al0, ar0, W1, al1, ar1, W2, al2, ar2):
    out, _ = run_gat(features, src, dst, W0, al0, ar0, W1, al1, ar1,
                     W2, al2, ar2, trace=False)
    return out


def run_gat(features, src, dst, W0, al0, ar0, W1, al1, ar1, W2, al2, ar2,
            trace=False):
    features = np.asarray(features, dtype=np.float32)
    n_nodes = features.shape[0]
    meta = preprocess(src, dst, n_nodes)
    NP, W, npc = meta["NP"], meta["W"], meta["npc"]

    Wm0, A0 = pack_weights(np.asarray(W0), al0, ar0)
    Wm1, A1 = pack_weights(np.asarray(W1), al1, ar1)
    Wm2, A2 = pack_weights(np.asarray(W2), al2, ar2)

    identF = np.eye(128, dtype=np.float32)
    identB = np.eye(128, dtype=np.float32).astype(NPBF)
    ones1 = np.ones((1, 128), dtype=np.float32)
    onescol = np.ones((128, 1), dtype=np.float32)

    in_maps = []
    for c in range(N_CORES):
        h_c = np.zeros((NP, 128), dtype=np.float32)
        h_c[:npc] = features[c * npc:(c + 1) * npc][meta["perm"][c]]
        in_maps.append({
            "hT0": np.ascontiguousarray(h_c.T).astype(NPBF),
            "idx": meta["idx_img"][c],
            "valid": meta["valid"][c],
            "W0": Wm0, "W1": Wm1, "W2": Wm2,
            "A0": A0, "A1": A1, "A2": A2,
            "identF": identF, "identB": identB,
            "ones1": ones1, "onescol": onescol,
        })

    nc = build_nc(meta)
    br = run_bass_kernel_spmd(nc, in_maps, list(range(N_CORES)), trace=trace)
    res = br.results

    out = np.empty((n_nodes, 128), dtype=np.float32)
    for c in range(N_CORES):
        o = np.asarray(res[c]["out"])
        out[c * npc:(c + 1) * npc] = o[np.argsort(meta["perm"][c])]
    return out, br


# revision 6
# speedup vs baseline: 1.1107x; 1.0285x over previous
"""GAT (3-layer, DGL-style) on 8 Trainium2 NeuronCores — v2 (bf16 tables).

Sharding: nodes across the 8 cores (6250 each, padded to 6272 = 49*128),
per-core nodes permuted by descending in-degree.  A "window" is 128 nodes;
a node is pinned to one SBUF partition lane of its window.  Per layer:

  Phase A (node side): featT = W^T @ h^T per window on PE (bf16), el/er via a
  small second matmul, build 512-byte gather-table rows
  [feat 128xbf16 | el 4xf32-bitcast | pad] in local DRAM.  The table is
  AllGathered in 4 window-quarter chunks interleaved with phase A so most of
  the collective hides behind compute.

  Phase B (edge side): per window, edge tiles of 128 edges = one in-edge per
  destination partition.  dma_gather fetches 512B source rows (int16 indices;
  the quarter-major table row order makes rows 0..24575 the "lo" half and
  24576..50175 the "hi" half, each addressable with int16).  All windows' lo
  tiles run first (partial sums flushed PSUM->SBUF), then all hi tiles (so
  the hi-half AllGather chunks hide behind the whole lo pass).  er[dst] is a
  per-partition constant.  exp(lrelu(s)-C) = max(exp(s-C), exp(0.2*s-C)) on
  ACT.  Messages (+ per-head exp columns, all bf16) are segment-summed by a
  bf16 identity-lhsT PE matmul accumulating into one PSUM bank per window.

C is a per-core bound lrelu(max el + max er) + 3 computed on device; shifting
exp by C instead of the per-segment max changes the reference's +1e-9 epsilon
term by < 1e-3 relative.
"""

import sys

sys.path.insert(0, "/opt/trn_rl_repo")

import numpy as np
import ml_dtypes

import concourse.bass as bass
import concourse.bacc as bacc
import concourse.mybir as mybir
import concourse.tile as tile
from concourse.bass_utils import run_bass_kernel_spmd

F32 = mybir.dt.float32
BF16 = mybir.dt.bfloat16
I16 = mybir.dt.int16
AF = mybir.ActivationFunctionType
OP = mybir.AluOpType
AX = mybir.AxisListType

NPBF = ml_dtypes.bfloat16

N_CORES = 8
DIM = 128
ROW_BF = 256           # table row stride in bf16 elems (512 B)
TBL_COLS = 136         # used cols: 128 feat bf16 + 4 el f32 (8 bf16 slots)
CAP = 16               # max tiles per dma_gather call
GRP = 8                # tiles per DVE/ACT group
NEG_SLOPE = 0.2
C_MARGIN = 3.0
HEADS = (4, 4, 1)

# window-quarter split for chunked AllGather (49 windows)
QWIN = (12, 12, 12, 13)


def _quarter_geometry(NP, W):
    qsize = [q * 128 for q in QWIN]                  # per-core rows per chunk
    qstart = np.cumsum([0] + qsize[:-1]).tolist()    # per-core pos offsets
    chunk_off = np.cumsum([0] + [N_CORES * s for s in qsize[:-1]]).tolist()
    return qsize, qstart, chunk_off


# ---------------------------------------------------------------------------
# Host-side preprocessing
# ---------------------------------------------------------------------------

def preprocess(src, dst, n_nodes):
    src = np.asarray(src).astype(np.int64)
    dst = np.asarray(dst).astype(np.int64)
    npc = n_nodes // N_CORES
    NP = ((npc + 127) // 128) * 128
    W = NP // 128
    assert W == sum(QWIN)
    qsize, qstart, chunk_off = _quarter_geometry(NP, W)
    HALF = chunk_off[2]
    assert HALF <= 32768 and (N_CORES * NP - HALF) <= 32768

    core = dst // npc
    local = dst - core * npc

    perm = []
    pos_of = np.empty(n_nodes, dtype=np.int64)
    for c in range(N_CORES):
        deg_c = np.bincount(local[core == c], minlength=npc)
        p = np.argsort(-deg_c, kind="stable")
        perm.append(p)
        inv = np.empty(npc, dtype=np.int64)
        inv[p] = np.arange(npc)
        pos_of[c * npc:(c + 1) * npc] = inv

    # quarter-major global table row for each node
    posq = pos_of  # per-core position 0..NP-1 (only first npc used per core)
    qidx = np.searchsorted(np.array(qstart[1:] + [NP]), posq, side="right")
    qs = np.array([qstart[q] for q in range(4)])[qidx]
    qz = np.array([qsize[q] for q in range(4)])[qidx]
    co = np.array([chunk_off[q] for q in range(4)])[qidx]
    node_core = np.arange(n_nodes) // npc
    row_of = co + node_core * qz + (posq - qs)

    seg_pos = pos_of[dst]
    wv = seg_pos // 128
    pv = seg_pos % 128
    half = (row_of[src] >= HALF).astype(np.int64)

    # occurrence rank within (core, seg, half)
    key = (core * NP + seg_pos) * 2 + half
    order = np.argsort(key, kind="stable")
    ks = key[order]
    starts = np.r_[0, np.flatnonzero(np.diff(ks)) + 1]
    gid = np.zeros(len(ks), dtype=np.int64)
    gid[starts[1:]] = 1
    gid = np.cumsum(gid)
    t_in = np.arange(len(ks)) - starts[gid]
    tv = np.empty(len(ks), dtype=np.int64)
    tv[order] = t_in

    cnt = np.bincount(key, minlength=N_CORES * NP * 2).reshape(
        N_CORES, W, 128, 2)
    T_lo = cnt[:, :, :, 0].max(axis=(0, 2)).astype(np.int64)
    T_hi = cnt[:, :, :, 1].max(axis=(0, 2)).astype(np.int64)
    assert (T_lo > 0).all() and (T_hi > 0).all()

    # lo pass over all windows first, then hi pass
    calls = []
    for hf, Tarr in ((0, T_lo), (1, T_hi)):
        for w in range(W):
            t = 0
            T = int(Tarr[w])
            while t < T:
                nt = min(CAP, T - t)
                calls.append((w, hf, nt))
                t += nt
    gtot = int(T_lo.sum() + T_hi.sum())
    icols = 8 * sum(nt for (_, _, nt) in calls)

    tile_off = np.zeros((W, 2), dtype=np.int64)
    acc = 0
    for w in range(W):
        tile_off[w, 0] = acc
        acc += T_lo[w]
        tile_off[w, 1] = acc
        acc += T_hi[w]

    idx_imgs, valids = [], []
    for c in range(N_CORES):
        m = core == c
        slots_idx = np.zeros((128, gtot), dtype=np.int64)
        slots_val = np.zeros((128, gtot), dtype=np.float32)
        g = tile_off[wv[m], half[m]] + tv[m]
        slots_idx[pv[m], g] = row_of[src[m]] - half[m] * HALF
        slots_val[pv[m], g] = 1.0
        img = np.zeros((16, icols), dtype=np.int16)
        colp = 0
        tile_ptr = {}
        for (w, hf, nt) in calls:
            t0 = tile_ptr.get((w, hf), 0)
            g0 = tile_off[w, hf] + t0
            part = slots_idx[:, g0:g0 + nt]          # [128, nt]
            flat = part.T.reshape(-1)                # j = t*128 + p
            img[:, colp:colp + nt * 8] = flat.reshape(nt * 8, 16).T
            colp += nt * 8
            tile_ptr[(w, hf)] = t0 + nt
        idx_imgs.append(np.ascontiguousarray(np.tile(img, (8, 1))))
        valids.append(slots_val.astype(NPBF))

    return dict(perm=perm, calls=calls, T_lo=T_lo, T_hi=T_hi,
                idx_img=idx_imgs, valid=valids, NP=NP, W=W, gtot=gtot,
                icols=icols, npc=npc, HALF=HALF,
                tile_off=tile_off, qsize=qsize, qstart=qstart,
                chunk_off=chunk_off)


def pack_weights(Wl, al, ar):
    H, Dh = Wl.shape[1], Wl.shape[2]
    Wm = np.ascontiguousarray(np.asarray(Wl, dtype=np.float32)
                              .reshape(Wl.shape[0], H * Dh))
    A = np.zeros((H * Dh, 8), dtype=np.float32)
    for h in range(H):
        A[h * Dh:(h + 1) * Dh, h] = np.asarray(al, dtype=np.float32)[h]
        A[h * Dh:(h + 1) * Dh, 4 + h] = np.asarray(ar, dtype=np.float32)[h]
    return Wm.astype(NPBF), A.astype(NPBF)


# ---------------------------------------------------------------------------
# Device kernel
# ---------------------------------------------------------------------------

def build_nc(meta):
    NP, W, gtot, icols = meta["NP"], meta["W"], meta["gtot"], meta["icols"]
    calls, HALF = meta["calls"], meta["HALF"]
    NTOT = N_CORES * NP
    tile_off = meta["tile_off"]
    qsize, qstart, chunk_off = meta["qsize"], meta["qstart"], meta["chunk_off"]
    qlastw = np.cumsum(QWIN) - 1  # windows 11, 23, 35, 48

    nc = bacc.Bacc(None, target_bir_lowering=False, debug=False,
                   num_devices=N_CORES, num_swdge_queues=4)

    hT0 = nc.declare_dram_parameter("hT0", [128, NP], BF16, isOutput=False)
    idx_p = nc.declare_dram_parameter("idx", [128, icols], I16, isOutput=False)
    val_p = nc.declare_dram_parameter("valid", [128, gtot], BF16,
                                      isOutput=False)
    Wp = [nc.declare_dram_parameter(f"W{l}", [128, 128], BF16, isOutput=False)
          for l in range(3)]
    Ap = [nc.declare_dram_parameter(f"A{l}", [128, 8], BF16, isOutput=False)
          for l in range(3)]
    identF_p = nc.declare_dram_parameter("identF", [128, 128], F32,
                                         isOutput=False)
    identB_p = nc.declare_dram_parameter("identB", [128, 128], BF16,
                                         isOutput=False)
    ones_p = nc.declare_dram_parameter("ones1", [1, 128], F32, isOutput=False)
    onescol_p = nc.declare_dram_parameter("onescol", [128, 1], F32,
                                          isOutput=False)
    out_p = nc.declare_dram_parameter("out", [NP, 128], F32, isOutput=True)

    with tile.TileContext(nc) as tc:
        with (
            tc.tile_pool(name="const", bufs=1) as constp,
            tc.tile_pool(name="persist", bufs=1) as pers,
            tc.tile_pool(name="featg", bufs=6) as fgp,
            tc.tile_pool(name="mext", bufs=6) as mxp,
            tc.tile_pool(name="small", bufs=8) as smp,
            tc.tile_pool(name="psum", bufs=3, space="PSUM") as psp,
            tc.tile_pool(name="psacc", bufs=2, space="PSUM") as psaccp,
            tc.tile_pool(name="dram", bufs=1, space="DRAM") as dramp,
        ):
            identF = constp.tile([128, 128], F32, tag="identF")
            nc.sync.dma_start(identF[:], identF_p[:, :])
            identB = constp.tile([128, 128], BF16, tag="identB")
            nc.sync.dma_start(identB[:], identB_p[:, :])
            ones1 = constp.tile([1, 128], F32, tag="ones1")
            nc.sync.dma_start(ones1[:], ones_p[:, :])
            onescol = constp.tile([128, 1], F32, tag="onescol")
            nc.sync.dma_start(onescol[:], onescol_p[:, :])
            Wt = [constp.tile([128, 128], BF16, tag=f"W{l}", name=f"Wt{l}")
                  for l in range(3)]
            At = [constp.tile([128, 8], BF16, tag=f"A{l}", name=f"At{l}")
                  for l in range(3)]
            for l in range(3):
                nc.sync.dma_start(Wt[l][:], Wp[l][:, :])
                nc.sync.dma_start(At[l][:], Ap[l][:, :])
            idx_sb = pers.tile([128, icols], I16, tag="idx")
            nc.sync.dma_start(idx_sb[:], idx_p[:, :])
            valid_sb = pers.tile([128, gtot], BF16, tag="valid")
            nc.sync.dma_start(valid_sb[:], val_p[:, :])

            hT = [pers.tile([128, W, 128], BF16, tag=f"hT{i}", name=f"hT{i}")
                  for i in range(2)]
            nc.sync.dma_start(hT[0][:, :, :],
                              hT0[:, :].rearrange("p (w n) -> p w n", w=W))

            elerB = pers.tile([128, W, 8], F32, tag="elerB")
            rowimg = pers.tile([128, W, TBL_COLS], BF16, tag="rowimg")
            accsb = pers.tile([128, W, 132], F32, tag="accsb")

            loc_tbl = dramp.tile([NP, ROW_BF], BF16, tag="loctbl")
            full_tbl = dramp.tile([NTOT, ROW_BF], BF16, tag="fulltbl")

            for layer in range(3):
                H = HEADS[layer]
                D = 128 // H
                hcur, hnext = hT[layer % 2], hT[(layer + 1) % 2]

                # ======== Phase A ========
                agq = 0
                for w in range(W):
                    featT_ps = psp.tile([128, 128], F32, tag="ps")
                    nc.tensor.matmul(featT_ps[:], Wt[layer][:],
                                     hcur[:, w, :], start=True, stop=True)
                    featT_sb = smp.tile([128, 128], BF16, tag="featT_sb")
                    nc.vector.tensor_copy(featT_sb[:], featT_ps[:])
                    elerT_ps = psp.tile([8, 128], F32, tag="ps")
                    nc.tensor.matmul(elerT_ps[:], At[layer][:], featT_sb[:],
                                     start=True, stop=True)
                    elerT_sb = smp.tile([8, 128], F32, tag="elerT_sb")
                    nc.vector.tensor_copy(elerT_sb[:], elerT_ps[:])
                    eler_ps = psp.tile([128, 8], F32, tag="ps")
                    nc.tensor.matmul(eler_ps[:], elerT_sb[:],
                                     identF[0:8, 0:8], is_transpose=True,
                                     start=True, stop=True)
                    nc.vector.tensor_copy(elerB[:, w, :], eler_ps[:])
                    feat_ps = psp.tile([128, 128], BF16, tag="psb")
                    nc.tensor.matmul(feat_ps[:], featT_sb[:], identB[:, :],
                                     is_transpose=True, start=True, stop=True)
                    nc.vector.tensor_copy(rowimg[:, w, 0:128], feat_ps[:])
                    nc.vector.tensor_copy(
                        rowimg[:, w, 128:TBL_COLS].bitcast(F32),
                        eler_ps[:, 0:4])
                    nc.sync.dma_start(
                        loc_tbl[:].rearrange("(w p) f -> w p f", p=128)
                        [w, :, 0:TBL_COLS],
                        rowimg[:, w, :])
                    if w == qlastw[agq]:
                        q = agq
                        nc.gpsimd.collective_compute(
                            "AllGather", OP.bypass,
                            replica_groups=[list(range(N_CORES))],
                            ins=[loc_tbl[qstart[q]:qstart[q] + qsize[q], :]
                                 .opt()],
                            outs=[full_tbl[chunk_off[q]:
                                           chunk_off[q] + N_CORES * qsize[q],
                                           :].opt()])
                        agq += 1

                # ---- -C = -(lrelu(max el + max er) + margin) ----
                mx = smp.tile([128, 2], F32, tag="mx")
                nc.vector.tensor_reduce(mx[:, 0:1], elerB[:, :, 0:H],
                                        axis=AX.XY, op=OP.max)
                nc.vector.tensor_reduce(mx[:, 1:2], elerB[:, :, 4:4 + H],
                                        axis=AX.XY, op=OP.max)
                mxT_ps = psp.tile([2, 128], F32, tag="ps")
                nc.tensor.matmul(mxT_ps[:], mx[:], identF[:, :],
                                 is_transpose=True, start=True, stop=True)
                mm = smp.tile([2, 1], F32, tag="mm")
                nc.vector.tensor_reduce(mm[:], mxT_ps[:, :], axis=AX.X,
                                        op=OP.max)
                s_ps = psp.tile([1, 1], F32, tag="ps")
                nc.tensor.matmul(s_ps[:], mm[:], onescol[0:2, 0:1],
                                 start=True, stop=True)
                cs = smp.tile([1, 4], F32, tag="cs")
                nc.vector.tensor_copy(cs[:, 0:1], s_ps[:])
                nc.vector.tensor_scalar(cs[:, 1:2], cs[:, 0:1], NEG_SLOPE,
                                        None, op0=OP.mult)
                nc.vector.tensor_tensor(cs[:, 2:3], cs[:, 0:1],
                                        cs[:, 1:2], op=OP.max)
                nc.vector.tensor_scalar(cs[:, 3:4], cs[:, 2:3], -1.0,
                                        -C_MARGIN, op0=OP.mult,
                                        op1=OP.add)
                negC_ps = psp.tile([128, 1], F32, tag="ps")
                nc.tensor.matmul(negC_ps[:], ones1[:], cs[:, 3:4],
                                 start=True, stop=True)
                negC = smp.tile([128, 1], F32, tag="negC")
                nc.vector.tensor_copy(negC[:], negC_ps[:])

                # zero the lo-pass partial sums (covers any empty windows)
                nc.vector.memset(accsb[:, :, :], 0.0)

                # ======== Phase B (lo pass over all windows, then hi) ====
                tbl_lo = full_tbl[0:HALF, :]
                tbl_hi = full_tbl[HALF:NTOT, :]
                colp = 0
                tile_ptr = {}
                cur_key = None
                acc_ps = None
                first_mm = True
                done = {}
                qn = 0
                for (w, hf, nt) in calls:
                    Thf = int((meta["T_lo"] if hf == 0 else meta["T_hi"])[w])
                    if (w, hf) != cur_key:
                        cur_key = (w, hf)
                        acc_ps = psaccp.tile([128, 132], F32, tag="acc")
                        first_mm = True
                        done[cur_key] = 0
                    t0 = tile_ptr.get((w, hf), 0)
                    tile_ptr[(w, hf)] = t0 + nt
                    g0 = int(tile_off[w, hf]) + t0

                    fg = fgp.tile([128, CAP, ROW_BF], BF16, tag="fg")
                    src_ap = tbl_lo if hf == 0 else tbl_hi
                    nc.gpsimd.dma_gather(
                        fg[:, 0:nt, :], src_ap,
                        idx_sb[:, colp:colp + nt * 8],
                        nt * 128, nt * 128, ROW_BF, elem_step=ROW_BF,
                        single_packet=False, queue_num=qn)
                    qn = (qn + 1) % 4
                    colp += nt * 8

                    t = 0
                    while t < nt:
                        g = min(GRP, nt - t)
                        sx = smp.tile([128, GRP, 4], F32, tag="sx")
                        ux = smp.tile([128, GRP, 4], BF16, tag="ux")
                        ex = smp.tile([128, GRP, 4], BF16, tag="exx")
                        er_b = (elerB[:, w, 4:4 + H].unsqueeze(1)
                                .broadcast_to([128, g, H]))
                        nc.vector.tensor_tensor(
                            sx[:, 0:g, 0:H],
                            fg[:, t:t + g, 128:128 + 2 * H].bitcast(F32),
                            er_b, op=OP.add)
                        nc.scalar.activation(ux[:, 0:g, 0:H], sx[:, 0:g, 0:H],
                                             AF.Exp, bias=negC[:, 0:1],
                                             scale=1.0)
                        nc.scalar.activation(ex[:, 0:g, 0:H], sx[:, 0:g, 0:H],
                                             AF.Exp, bias=negC[:, 0:1],
                                             scale=NEG_SLOPE)
                        val_b = (valid_sb[:, g0 + t:g0 + t + g].unsqueeze(2)
                                 .broadcast_to([128, g, H]))
                        nc.vector.scalar_tensor_tensor(
                            ex[:, 0:g, 0:H], ux[:, 0:g, 0:H], 1.0,
                            ex[:, 0:g, 0:H], op0=OP.mult, op1=OP.max)
                        nc.vector.tensor_tensor(ex[:, 0:g, 0:H],
                                                ex[:, 0:g, 0:H], val_b,
                                                op=OP.mult)
                        mext = mxp.tile([128, GRP, 132], BF16, tag="mext")
                        ex_b = (ex[:, 0:g, 0:H].unsqueeze(3)
                                .broadcast_to([128, g, H, D]))
                        nc.vector.tensor_tensor(
                            mext[:, 0:g, 0:128]
                            .rearrange("p g (h d) -> p g h d", h=H),
                            fg[:, t:t + g, 0:128]
                            .rearrange("p g (h d) -> p g h d", h=H),
                            ex_b, op=OP.mult)
                        nc.vector.tensor_copy(mext[:, 0:g, 128:128 + H],
                                              ex[:, 0:g, 0:H])
                        for k in range(g):
                            done[cur_key] += 1
                            nc.tensor.matmul(
                                acc_ps[:, 0:128 + H], identB[:, :],
                                mext[:, k, 0:128 + H],
                                start=first_mm,
                                stop=(done[cur_key] == Thf))
                            first_mm = False
                        t += g

                    if done[cur_key] != Thf:
                        continue
                    if hf == 0:
                        # flush lo partial into SBUF
                        nc.vector.tensor_copy(accsb[:, w, 0:128 + H],
                                              acc_ps[:, 0:128 + H])
                        continue
                    # hi pass window epilogue: combine, normalize, emit
                    accf = smp.tile([128, 132], F32, tag="accf")
                    nc.vector.tensor_tensor(accf[:, 0:128 + H],
                                            acc_ps[:, 0:128 + H],
                                            accsb[:, w, 0:128 + H],
                                            op=OP.add)
                    dn = smp.tile([128, 8], F32, tag="dn")
                    nc.vector.tensor_scalar(dn[:, 0:H],
                                            accf[:, 128:128 + H],
                                            1e-9, None, op0=OP.add)
                    nc.vector.reciprocal(dn[:, 4:4 + H], dn[:, 0:H])
                    hsb = smp.tile([128, 128], F32, tag="hsb")
                    rec_b = (dn[:, 4:4 + H].unsqueeze(2)
                             .broadcast_to([128, H, D]))
                    nc.vector.tensor_tensor(
                        hsb[:].rearrange("p (h d) -> p h d", h=H),
                        accf[:, 0:128]
                        .rearrange("p (h d) -> p h d", h=H),
                        rec_b, op=OP.mult)
                    if layer < 2:
                        hT_ps = psp.tile([128, 128], F32, tag="ps")
                        nc.tensor.matmul(hT_ps[:], hsb[:], identF[:, :],
                                         is_transpose=True,
                                         start=True, stop=True)
                        nc.scalar.activation(hnext[:, w, :], hT_ps[:],
                                             AF.Relu)
                    else:
                        nc.sync.dma_start(
                            out_p[:, :].rearrange("(w p) f -> w p f",
                                                  p=128)[w, :, :],
                            hsb[:])
    nc.finalize()
    return nc


# ---------------------------------------------------------------------------
# Entry point
# ---------------------------------------------------------------------------

def kernel(features, src, dst, W0, al0, ar0, W1, al1, ar1, W2, al2, ar2):
    out, _ = run_gat(features, src, dst, W0, al0, ar0, W1, al1, ar1,
                     W2, al2, ar2, trace=False)
    return out


def run_gat(features, src, dst, W0, al0, ar0, W1, al1, ar1, W2, al2, ar2,
            trace=False):
    features = np.asarray(features, dtype=np.float32)
    n_nodes = features.shape[0]
    meta = preprocess(src, dst, n_nodes)
    NP, W, npc = meta["NP"], meta["W"], meta["npc"]

    Wm0, A0 = pack_weights(np.asarray(W0), al0, ar0)
    Wm1, A1 = pack_weights(np.asarray(W1), al1, ar1)
    Wm2, A2 = pack_weights(np.asarray(W2), al2, ar2)

    identF = np.eye(128, dtype=np.float32)
    identB = np.eye(128, dtype=np.float32).astype(NPBF)
    ones1 = np.ones((1, 128), dtype=np.float32)
    onescol = np.ones((128, 1), dtype=np.float32)

    in_maps = []
    for c in range(N_CORES):
        h_c = np.zeros((NP, 128), dtype=np.float32)
        h_c[:npc] = features[c * npc:(c + 1) * npc][meta["perm"][c]]
        in_maps.append({
            "hT0": np.ascontiguousarray(h_c.T).astype(NPBF),
            "idx": meta["idx_img"][c],
            "valid": meta["valid"][c],
            "W0": Wm0, "W1": Wm1, "W2": Wm2,
            "A0": A0, "A1": A1, "A2": A2,
            "identF": identF, "identB": identB,
            "ones1": ones1, "onescol": onescol,
        })

    nc = build_nc(meta)
    br = run_bass_kernel_spmd(nc, in_maps, list(range(N_CORES)), trace=trace)
    res = br.results

    out = np.empty((n_nodes, 128), dtype=np.float32)
    for c in range(N_CORES):
        o = np.asarray(res[c]["out"])
        out[c * npc:(c + 1) * npc] = o[np.argsort(meta["perm"][c])]
    return out, br


# revision 7
# speedup vs baseline: 2.7071x; 2.4374x over previous
"""GAT (3-layer, DGL-style) on 8 Trainium2 NeuronCores — v4.

Sharding: nodes across the 8 cores (6250 each, padded to 6272 = 49*128),
per-core nodes permuted by descending in-degree.  A "window" is 128 nodes;
a node is pinned to one SBUF partition lane of its window.  Per layer:

  Phase A (node side): featT = W^T @ h^T per window on PE (bf16), el/er via a
  small second matmul, build 512-byte gather-table rows
  [feat 128xbf16 | el 4xf32] in local DRAM.  The quarter-major table is
  AllGathered in 4 window-quarter chunks interleaved with phase A.

  Phase B (edge side): per window, edge tiles of 128 edges = one in-edge per
  destination partition.  dma_gather fetches 512B source rows.  int16 index
  range is handled with 7 overlapping base windows into the table (stride
  6144 rows, each covering 32768 rows); each tile is typed by base and each
  edge is assigned to a compatible tile host-side with an
  earliest-deadline-first pass, which keeps the tile count within ~2% of the
  per-lane max in-degree bound (no lo/hi table split).  er[dst] is a
  per-partition constant.  exp(lrelu(s)-C) = max(exp(s-C), exp(0.2*s-C)) on
  ACT.  Messages (+ per-head exp columns, bf16) are segment-summed by a bf16
  identity-lhsT PE matmul accumulating into one PSUM bank per window.

C is a per-core bound lrelu(max el + max er) + 3 computed on device; shifting
exp by C instead of the per-segment max changes the reference's +1e-9 epsilon
term by < 1e-3 relative.
"""

import sys

sys.path.insert(0, "/opt/trn_rl_repo")

import numpy as np
import ml_dtypes

import concourse.bass as bass
import concourse.bacc as bacc
import concourse.mybir as mybir
import concourse.tile as tile
from concourse.bass_utils import run_bass_kernel_spmd

F32 = mybir.dt.float32
BF16 = mybir.dt.bfloat16
I16 = mybir.dt.int16
AF = mybir.ActivationFunctionType
OP = mybir.AluOpType
AX = mybir.AxisListType

NPBF = ml_dtypes.bfloat16

N_CORES = 8
DIM = 128
ROW_BF = 256           # table row stride in bf16 elems (512 B)
TBL_COLS = 136         # used cols: 128 feat bf16 + 4 el f32 (8 bf16 slots)
CAP = 16               # max tiles per dma_gather call
GRP = 8                # tiles per DVE/ACT group
NBASE = 7              # overlapping int16 index bases
BSTEP = 6144           # base stride in table rows
NEG_SLOPE = 0.2
C_MARGIN = 3.0
HEADS = (4, 4, 1)

QWIN = (12, 12, 12, 13)  # window-quarter split for chunked AllGather


# ---------------------------------------------------------------------------
# Host-side preprocessing
# ---------------------------------------------------------------------------

def preprocess(src, dst, n_nodes):
    src = np.asarray(src).astype(np.int64)
    dst = np.asarray(dst).astype(np.int64)
    npc = n_nodes // N_CORES
    NP = ((npc + 127) // 128) * 128
    W = NP // 128
    assert W == sum(QWIN)
    qsize = [q * 128 for q in QWIN]
    qstart = np.cumsum([0] + qsize[:-1])
    chunk_off = np.cumsum([0] + [N_CORES * s for s in qsize[:-1]])

    core = dst // npc
    local = dst - core * npc

    perm = []
    pos_of = np.empty(n_nodes, dtype=np.int64)
    for c in range(N_CORES):
        deg_c = np.bincount(local[core == c], minlength=npc)
        p = np.argsort(-deg_c, kind="stable")
        perm.append(p)
        inv = np.empty(npc, dtype=np.int64)
        inv[p] = np.arange(npc)
        pos_of[c * npc:(c + 1) * npc] = inv

    # quarter-major global table row for each node
    qidx = np.searchsorted(np.array(list(qstart[1:]) + [NP]), pos_of,
                           side="right")
    qs = np.array(qstart)[qidx]
    qz = np.array(qsize)[qidx]
    co = np.array(chunk_off)[qidx]
    node_core = np.arange(n_nodes) // npc
    row_of = co + node_core * qz + (pos_of - qs)

    seg_pos = pos_of[dst]
    wv = seg_pos // 128
    pv = seg_pos % 128

    # int16 base windows: edge with table row r usable from base k iff
    # BSTEP*k <= r <= BSTEP*k + 32767
    r = row_of[src]
    lo = np.maximum(0, -(-(r - 32767) // BSTEP))
    hi = np.minimum(NBASE - 1, r // BSTEP)
    lane = (core * W + wv) * 128 + pv

    # per-lane interval counts -> per-window typed-tile quotas T[w, k]
    ikey = lane * (NBASE * NBASE) + lo * NBASE + hi
    cntI = np.bincount(ikey, minlength=N_CORES * W * 128 * NBASE * NBASE)
    cntI = cntI.reshape(-1, NBASE, NBASE)
    crev = cntI[:, ::-1, :].cumsum(axis=1)[:, ::-1, :]
    cc = (crev.cumsum(axis=2).reshape(N_CORES, W, 128, NBASE, NBASE)
          .max(axis=(0, 2)))
    T = np.zeros((W, NBASE), np.int64)
    for w in range(W):
        for k in range(NBASE):
            best = 0
            for a in range(k + 1):
                best = max(best, cc[w, a, k] - T[w, a:k].sum())
            T[w, k] = best

    # earliest-deadline-first edge -> (base, slot) assignment
    E = len(src)
    assigned = np.zeros(E, bool)
    kass = np.full(E, -1, np.int64)
    tv = np.full(E, -1, np.int64)
    w_of_lane = (np.arange(N_CORES * W * 128) // 128) % W
    for k in range(NBASE):
        elig = (~assigned) & (lo <= k) & (k <= hi)
        idxs = np.flatnonzero(elig)
        order = idxs[np.lexsort((hi[idxs], lane[idxs]))]
        lane_o = lane[order]
        newg = np.r_[True, lane_o[1:] != lane_o[:-1]]
        starts = np.flatnonzero(newg)
        gidx = np.cumsum(newg) - 1
        cumc = np.arange(len(order)) - starts[gidx]
        take = cumc < T[w_of_lane[lane_o], k]
        sel = order[take]
        assigned[sel] = True
        kass[sel] = k
        tv[sel] = cumc[take]
    assert assigned.all()

    calls = []
    for w in range(W):
        for k in range(NBASE):
            t = 0
            Twk = int(T[w, k])
            while t < Twk:
                nt = min(CAP, Twk - t)
                calls.append((w, k, nt))
                t += nt
    gtot = int(T.sum())
    icols = 8 * sum(nt for (_, _, nt) in calls)

    tile_off = np.zeros((W, NBASE), dtype=np.int64)
    acc = 0
    for w in range(W):
        for k in range(NBASE):
            tile_off[w, k] = acc
            acc += T[w, k]

    idx_imgs, valids = [], []
    for c in range(N_CORES):
        m = core == c
        slots_idx = np.zeros((128, gtot), dtype=np.int64)
        slots_val = np.zeros((128, gtot), dtype=np.float32)
        g = tile_off[wv[m], kass[m]] + tv[m]
        slots_idx[pv[m], g] = r[m] - kass[m] * BSTEP
        slots_val[pv[m], g] = 1.0
        img = np.zeros((16, icols), dtype=np.int16)
        colp = 0
        tile_ptr = {}
        for (w, k, nt) in calls:
            t0 = tile_ptr.get((w, k), 0)
            g0 = tile_off[w, k] + t0
            part = slots_idx[:, g0:g0 + nt]          # [128, nt]
            flat = part.T.reshape(-1)                # j = t*128 + p
            img[:, colp:colp + nt * 8] = flat.reshape(nt * 8, 16).T
            colp += nt * 8
            tile_ptr[(w, k)] = t0 + nt
        idx_imgs.append(np.ascontiguousarray(np.tile(img, (8, 1))))
        valids.append(slots_val.astype(NPBF))

    return dict(perm=perm, calls=calls, T=T,
                idx_img=idx_imgs, valid=valids, NP=NP, W=W, gtot=gtot,
                icols=icols, npc=npc, tile_off=tile_off,
                qsize=qsize, qstart=list(qstart), chunk_off=list(chunk_off))


def pack_weights(Wl, al, ar):
    H, Dh = Wl.shape[1], Wl.shape[2]
    Wm = np.ascontiguousarray(np.asarray(Wl, dtype=np.float32)
                              .reshape(Wl.shape[0], H * Dh))
    A = np.zeros((H * Dh, 8), dtype=np.float32)
    for h in range(H):
        A[h * Dh:(h + 1) * Dh, h] = np.asarray(al, dtype=np.float32)[h]
        A[h * Dh:(h + 1) * Dh, 4 + h] = np.asarray(ar, dtype=np.float32)[h]
    return Wm.astype(NPBF), A.astype(NPBF)


# ---------------------------------------------------------------------------
# Device kernel
# ---------------------------------------------------------------------------

def build_nc(meta):
    NP, W, gtot, icols = meta["NP"], meta["W"], meta["gtot"], meta["icols"]
    calls = meta["calls"]
    NTOT = N_CORES * NP
    tile_off = meta["tile_off"]
    T = meta["T"]
    qsize, qstart, chunk_off = meta["qsize"], meta["qstart"], meta["chunk_off"]
    qlastw = np.cumsum(QWIN) - 1

    nc = bacc.Bacc(None, target_bir_lowering=False, debug=False,
                   num_devices=N_CORES, num_swdge_queues=4)

    hT0 = nc.declare_dram_parameter("hT0", [128, NP], BF16, isOutput=False)
    idx_p = nc.declare_dram_parameter("idx", [128, icols], I16, isOutput=False)
    val_p = nc.declare_dram_parameter("valid", [128, gtot], BF16,
                                      isOutput=False)
    Wp = [nc.declare_dram_parameter(f"W{l}", [128, 128], BF16, isOutput=False)
          for l in range(3)]
    Ap = [nc.declare_dram_parameter(f"A{l}", [128, 8], BF16, isOutput=False)
          for l in range(3)]
    identF_p = nc.declare_dram_parameter("identF", [128, 128], F32,
                                         isOutput=False)
    identB_p = nc.declare_dram_parameter("identB", [128, 128], BF16,
                                         isOutput=False)
    ones_p = nc.declare_dram_parameter("ones1", [1, 128], F32, isOutput=False)
    onescol_p = nc.declare_dram_parameter("onescol", [128, 1], F32,
                                          isOutput=False)
    out_p = nc.declare_dram_parameter("out", [NP, 128], F32, isOutput=True)

    with tile.TileContext(nc) as tc:
        with (
            tc.tile_pool(name="const", bufs=1) as constp,
            tc.tile_pool(name="persist", bufs=1) as pers,
            tc.tile_pool(name="featg", bufs=6) as fgp,
            tc.tile_pool(name="mext", bufs=6) as mxp,
            tc.tile_pool(name="small", bufs=8) as smp,
            tc.tile_pool(name="psum", bufs=3, space="PSUM") as psp,
            tc.tile_pool(name="psacc", bufs=2, space="PSUM") as psaccp,
            tc.tile_pool(name="dram", bufs=1, space="DRAM") as dramp,
        ):
            identF = constp.tile([128, 128], F32, tag="identF")
            nc.sync.dma_start(identF[:], identF_p[:, :])
            identB = constp.tile([128, 128], BF16, tag="identB")
            nc.sync.dma_start(identB[:], identB_p[:, :])
            ones1 = constp.tile([1, 128], F32, tag="ones1")
            nc.sync.dma_start(ones1[:], ones_p[:, :])
            onescol = constp.tile([128, 1], F32, tag="onescol")
            nc.sync.dma_start(onescol[:], onescol_p[:, :])
            Wt = [constp.tile([128, 128], BF16, tag=f"W{l}", name=f"Wt{l}")
                  for l in range(3)]
            At = [constp.tile([128, 8], BF16, tag=f"A{l}", name=f"At{l}")
                  for l in range(3)]
            for l in range(3):
                nc.sync.dma_start(Wt[l][:], Wp[l][:, :])
                nc.sync.dma_start(At[l][:], Ap[l][:, :])
            idx_sb = pers.tile([128, icols], I16, tag="idx")
            nc.sync.dma_start(idx_sb[:], idx_p[:, :])
            valid_sb = pers.tile([128, gtot], BF16, tag="valid")
            nc.sync.dma_start(valid_sb[:], val_p[:, :])

            hT = [pers.tile([128, W, 128], BF16, tag=f"hT{i}", name=f"hT{i}")
                  for i in range(2)]
            nc.sync.dma_start(hT[0][:, :, :],
                              hT0[:, :].rearrange("p (w n) -> p w n", w=W))

            elerB = pers.tile([128, W, 8], F32, tag="elerB")
            rowimg = pers.tile([128, W, TBL_COLS], BF16, tag="rowimg")

            loc_tbl = dramp.tile([NP, ROW_BF], BF16, tag="loctbl")
            full_tbl = dramp.tile([NTOT, ROW_BF], BF16, tag="fulltbl")

            for layer in range(3):
                H = HEADS[layer]
                D = 128 // H
                hcur, hnext = hT[layer % 2], hT[(layer + 1) % 2]

                # ======== Phase A ========
                agq = 0
                for w in range(W):
                    featT_ps = psp.tile([128, 128], F32, tag="ps")
                    nc.tensor.matmul(featT_ps[:], Wt[layer][:],
                                     hcur[:, w, :], start=True, stop=True)
                    featT_sb = smp.tile([128, 128], BF16, tag="featT_sb")
                    nc.vector.tensor_copy(featT_sb[:], featT_ps[:])
                    elerT_ps = psp.tile([8, 128], F32, tag="ps")
                    nc.tensor.matmul(elerT_ps[:], At[layer][:], featT_sb[:],
                                     start=True, stop=True)
                    elerT_sb = smp.tile([8, 128], F32, tag="elerT_sb")
                    nc.vector.tensor_copy(elerT_sb[:], elerT_ps[:])
                    eler_ps = psp.tile([128, 8], F32, tag="ps")
                    nc.tensor.matmul(eler_ps[:], elerT_sb[:],
                                     identF[0:8, 0:8], is_transpose=True,
                                     start=True, stop=True)
                    nc.vector.tensor_copy(elerB[:, w, :], eler_ps[:])
                    feat_ps = psp.tile([128, 128], BF16, tag="psb")
                    nc.tensor.matmul(feat_ps[:], featT_sb[:], identB[:, :],
                                     is_transpose=True, start=True, stop=True)
                    nc.vector.tensor_copy(rowimg[:, w, 0:128], feat_ps[:])
                    nc.vector.tensor_copy(
                        rowimg[:, w, 128:TBL_COLS].bitcast(F32),
                        eler_ps[:, 0:4])
                    nc.sync.dma_start(
                        loc_tbl[:].rearrange("(w p) f -> w p f", p=128)
                        [w, :, 0:TBL_COLS],
                        rowimg[:, w, :])
                    if w == qlastw[agq]:
                        q = agq
                        nc.gpsimd.collective_compute(
                            "AllGather", OP.bypass,
                            replica_groups=[list(range(N_CORES))],
                            ins=[loc_tbl[qstart[q]:qstart[q] + qsize[q], :]
                                 .opt()],
                            outs=[full_tbl[chunk_off[q]:
                                           chunk_off[q] + N_CORES * qsize[q],
                                           :].opt()])
                        agq += 1

                # ---- -C = -(lrelu(max el + max er) + margin) ----
                mx = smp.tile([128, 2], F32, tag="mx")
                nc.vector.tensor_reduce(mx[:, 0:1], elerB[:, :, 0:H],
                                        axis=AX.XY, op=OP.max)
                nc.vector.tensor_reduce(mx[:, 1:2], elerB[:, :, 4:4 + H],
                                        axis=AX.XY, op=OP.max)
                mxT_ps = psp.tile([2, 128], F32, tag="ps")
                nc.tensor.matmul(mxT_ps[:], mx[:], identF[:, :],
                                 is_transpose=True, start=True, stop=True)
                mm = smp.tile([2, 1], F32, tag="mm")
                nc.vector.tensor_reduce(mm[:], mxT_ps[:, :], axis=AX.X,
                                        op=OP.max)
                s_ps = psp.tile([1, 1], F32, tag="ps")
                nc.tensor.matmul(s_ps[:], mm[:], onescol[0:2, 0:1],
                                 start=True, stop=True)
                cs = smp.tile([1, 4], F32, tag="cs")
                nc.vector.tensor_copy(cs[:, 0:1], s_ps[:])
                nc.vector.tensor_scalar(cs[:, 1:2], cs[:, 0:1], NEG_SLOPE,
                                        None, op0=OP.mult)
                nc.vector.tensor_tensor(cs[:, 2:3], cs[:, 0:1],
                                        cs[:, 1:2], op=OP.max)
                nc.vector.tensor_scalar(cs[:, 3:4], cs[:, 2:3], -1.0,
                                        -C_MARGIN, op0=OP.mult,
                                        op1=OP.add)
                negC_ps = psp.tile([128, 1], F32, tag="ps")
                nc.tensor.matmul(negC_ps[:], ones1[:], cs[:, 3:4],
                                 start=True, stop=True)
                negC = smp.tile([128, 1], F32, tag="negC")
                nc.vector.tensor_copy(negC[:], negC_ps[:])

                # ======== Phase B ========
                src_aps = [full_tbl[BSTEP * k:
                                    min(BSTEP * k + 32768, NTOT), :]
                           for k in range(NBASE)]
                colp = 0
                tile_ptr = {}
                cur_w = -1
                acc_ps = None
                first_mm = True
                done_w = {w: 0 for w in range(W)}
                ntiles_w = {w: int(T[w].sum()) for w in range(W)}
                qn = 0
                for (w, k, nt) in calls:
                    if w != cur_w:
                        cur_w = w
                        acc_ps = psaccp.tile([128, 132], F32, tag="acc")
                        first_mm = True
                    t0 = tile_ptr.get((w, k), 0)
                    tile_ptr[(w, k)] = t0 + nt
                    g0 = int(tile_off[w, k]) + t0

                    fg = fgp.tile([128, CAP, ROW_BF], BF16, tag="fg")
                    nc.gpsimd.dma_gather(
                        fg[:, 0:nt, :], src_aps[k],
                        idx_sb[:, colp:colp + nt * 8],
                        nt * 128, nt * 128, ROW_BF, elem_step=ROW_BF,
                        single_packet=False, queue_num=qn)
                    qn = (qn + 1) % 4
                    colp += nt * 8

                    t = 0
                    while t < nt:
                        g = min(GRP, nt - t)
                        sx = smp.tile([128, GRP, 4], F32, tag="sx")
                        ux = smp.tile([128, GRP, 4], BF16, tag="ux")
                        ex = smp.tile([128, GRP, 4], BF16, tag="exx")
                        er_b = (elerB[:, w, 4:4 + H].unsqueeze(1)
                                .broadcast_to([128, g, H]))
                        nc.vector.tensor_tensor(
                            sx[:, 0:g, 0:H],
                            fg[:, t:t + g, 128:128 + 2 * H].bitcast(F32),
                            er_b, op=OP.add)
                        nc.scalar.activation(ux[:, 0:g, 0:H], sx[:, 0:g, 0:H],
                                             AF.Exp, bias=negC[:, 0:1],
                                             scale=1.0)
                        nc.scalar.activation(ex[:, 0:g, 0:H], sx[:, 0:g, 0:H],
                                             AF.Exp, bias=negC[:, 0:1],
                                             scale=NEG_SLOPE)
                        val_b = (valid_sb[:, g0 + t:g0 + t + g].unsqueeze(2)
                                 .broadcast_to([128, g, H]))
                        nc.vector.scalar_tensor_tensor(
                            ex[:, 0:g, 0:H], ux[:, 0:g, 0:H], 1.0,
                            ex[:, 0:g, 0:H], op0=OP.mult, op1=OP.max)
                        nc.vector.tensor_tensor(ex[:, 0:g, 0:H],
                                                ex[:, 0:g, 0:H], val_b,
                                                op=OP.mult)
                        mext = mxp.tile([128, GRP, 132], BF16, tag="mext")
                        ex_b = (ex[:, 0:g, 0:H].unsqueeze(3)
                                .broadcast_to([128, g, H, D]))
                        nc.vector.tensor_tensor(
                            mext[:, 0:g, 0:128]
                            .rearrange("p g (h d) -> p g h d", h=H),
                            fg[:, t:t + g, 0:128]
                            .rearrange("p g (h d) -> p g h d", h=H),
                            ex_b, op=OP.mult)
                        nc.vector.tensor_copy(mext[:, 0:g, 128:128 + H],
                                              ex[:, 0:g, 0:H])
                        for kk in range(g):
                            done_w[w] += 1
                            nc.tensor.matmul(
                                acc_ps[:, 0:128 + H], identB[:, :],
                                mext[:, kk, 0:128 + H],
                                start=first_mm,
                                stop=(done_w[w] == ntiles_w[w]))
                            first_mm = False
                        t += g

                    if done_w[w] != ntiles_w[w]:
                        continue
                    dn = smp.tile([128, 8], F32, tag="dn")
                    nc.vector.tensor_scalar(dn[:, 0:H],
                                            acc_ps[:, 128:128 + H],
                                            1e-9, None, op0=OP.add)
                    nc.vector.reciprocal(dn[:, 4:4 + H], dn[:, 0:H])
                    hsb = smp.tile([128, 128], F32, tag="hsb")
                    rec_b = (dn[:, 4:4 + H].unsqueeze(2)
                             .broadcast_to([128, H, D]))
                    nc.vector.tensor_tensor(
                        hsb[:].rearrange("p (h d) -> p h d", h=H),
                        acc_ps[:, 0:128]
                        .rearrange("p (h d) -> p h d", h=H),
                        rec_b, op=OP.mult)
                    if layer < 2:
                        hT_ps = psp.tile([128, 128], F32, tag="ps")
                        nc.tensor.matmul(hT_ps[:], hsb[:], identF[:, :],
                                         is_transpose=True,
                                         start=True, stop=True)
                        nc.scalar.activation(hnext[:, w, :], hT_ps[:],
                                             AF.Relu)
                    else:
                        nc.sync.dma_start(
                            out_p[:, :].rearrange("(w p) f -> w p f",
                                                  p=128)[w, :, :],
                            hsb[:])
    nc.finalize()
    return nc


# ---------------------------------------------------------------------------
# Entry point
# ---------------------------------------------------------------------------

def kernel(features, src, dst, W0, al0, ar0, W1, al1, ar1, W2, al2, ar2):
    out, _ = run_gat(features, src, dst, W0, al0, ar0, W1, al1, ar1,
                     W2, al2, ar2, trace=False)
    return out


def run_gat(features, src, dst, W0, al0, ar0, W1, al1, ar1, W2, al2, ar2,
            trace=False):
    features = np.asarray(features, dtype=np.float32)
    n_nodes = features.shape[0]
    meta = preprocess(src, dst, n_nodes)
    NP, W, npc = meta["NP"], meta["W"], meta["npc"]

    Wm0, A0 = pack_weights(np.asarray(W0), al0, ar0)
    Wm1, A1 = pack_weights(np.asarray(W1), al1, ar1)
    Wm2, A2 = pack_weights(np.asarray(W2), al2, ar2)

    identF = np.eye(128, dtype=np.float32)
    identB = np.eye(128, dtype=np.float32).astype(NPBF)
    ones1 = np.ones((1, 128), dtype=np.float32)
    onescol = np.ones((128, 1), dtype=np.float32)

    in_maps = []
    for c in range(N_CORES):
        h_c = np.zeros((NP, 128), dtype=np.float32)
        h_c[:npc] = features[c * npc:(c + 1) * npc][meta["perm"][c]]
        in_maps.append({
            "hT0": np.ascontiguousarray(h_c.T).astype(NPBF),
            "idx": meta["idx_img"][c],
            "valid": meta["valid"][c],
            "W0": Wm0, "W1": Wm1, "W2": Wm2,
            "A0": A0, "A1": A1, "A2": A2,
            "identF": identF, "identB": identB,
            "ones1": ones1, "onescol": onescol,
        })

    nc = build_nc(meta)
    br = run_bass_kernel_spmd(nc, in_maps, list(range(N_CORES)), trace=trace)
    res = br.results

    out = np.empty((n_nodes, 128), dtype=np.float32)
    for c in range(N_CORES):
        o = np.asarray(res[c]["out"])
        out[c * npc:(c + 1) * npc] = o[np.argsort(meta["perm"][c])]
    return out, br
